# revision 5
# baseline (speedup 1.0000x reference)
"""GNN message-passing kernel for Trainium2 (8 NeuronCores, batch-parallel).

Computation (per reference):
    norm_adj = adjacency * dinv * dinv.T + I            [10,10]   (host, O(100) flops)
    support  = einsum('bcf,fo->bco', x, kernel)         [B,C,O]
    out      = elu(einsum('ij,bjo->bio', norm_adj, support) + bias)
    out      = (out - mean) * rsqrt(var+eps) * gamma + beta

Strategy: the channel mix commutes with the dense matmul
(norm_adj @ (x @ K) == (norm_adj @ x) @ K), and is only ~0.5% of the
FLOPs, so the host pre-mixes y = norm_adj @ x, pre-transposes it to the
[f, rows] layout the PE needs, and ships it in bf16 (half the DMA, full
PE rate, ~1e-3 relative error).  Each core then runs a single streaming
matmul at the fp22 roofline:

  outT[o, rows] += K[f,o].T @ yT[f, rows]    (bf16 x bf16 -> fp32 PSUM)

with a fused epilogue on ACT/DVE with per-partition (o) params:
  elu(z) = min(exp(z), relu(z)+1) - 1 (exact), then folded BN affine.
Output is stored transposed [O, rows] in bf16; host casts/transposes
while unsharding.
"""

from contextlib import ExitStack

import numpy as np
import ml_dtypes

import concourse.bass as bass
import concourse.bacc as bacc
import concourse.mybir as mybir
import concourse.tile as tile
from concourse.bass_utils import run_bass_kernel_spmd

F32 = mybir.dt.float32
BF16 = mybir.dt.bfloat16
NP_BF16 = ml_dtypes.bfloat16
ALU = mybir.AluOpType
ACTF = mybir.ActivationFunctionType

P = 128
BN_EPS = 1e-3
N_CORES = 8
C = 10  # channels


def build_nc(rows, F, O, panel=512, n_cores=N_CORES, repeats=1):
    """Build the per-core Bass program. rows = local (b,c) rows, F/O = feat dims.

    repeats>1 replays the whole computation (for timing amplification)."""
    assert rows % panel == 0
    n_panels = rows // panel
    FT, OT = F // P, O // P

    nc = bacc.Bacc(
        "TRN2",
        target_bir_lowering=False,
        debug=False,
        enable_asserts=False,
        num_devices=n_cores,
    )
    # yt packs the pre-mixed, pre-transposed activations: yt[p, fb, r] =
    # y[r, fb*128+p] so a panel slice is one strided DMA into SBUF layout.
    yt_d = nc.dram_tensor("yt", [P, FT, rows], BF16, kind="ExternalInput").ap()
    k_d = nc.dram_tensor("kern", [F, O], BF16, kind="ExternalInput").ap()
    # blob cols: [0:OT]=bias_t, [OT:2OT]=scale_t, [2OT:3OT]=shift2_t (per-partition o)
    blob_d = nc.dram_tensor("blob", [P, 3 * OT], F32, kind="ExternalInput").ap()
    outT_d = nc.dram_tensor("outT", [O, rows], BF16, kind="ExternalOutput").ap()

    with tile.TileContext(nc) as tc, ExitStack() as ctx:
        const = ctx.enter_context(tc.tile_pool(name="const", bufs=1))
        blob = const.tile([P, 3 * OT], F32, name="blob")
        nc.sync.dma_start(blob, blob_d)
        kb = [const.tile([P, O], BF16, name=f"kb{fb}", tag=f"kb{fb}") for fb in range(FT)]
        for fb in range(FT):
            nc.scalar.dma_start(kb[fb], k_d[fb * P : (fb + 1) * P, :])

        ypool = ctx.enter_context(tc.tile_pool(name="ypool", bufs=3))
        mainps = ctx.enter_context(tc.tile_pool(name="mainps", bufs=4, space="PSUM"))
        tmp = ctx.enter_context(tc.tile_pool(name="tmp", bufs=3))

        for rep in range(repeats):
          for pi in range(n_panels):
            r0 = pi * panel
            yt = ypool.tile([P, FT, panel], BF16, name=f"r{rep}_y{pi}", tag="yt")
            nc.sync.dma_start(yt, yt_d[:, :, r0 : r0 + panel])
            for ot in range(OT):
                ps = mainps.tile([P, panel], F32, name=f"r{rep}_ps_{pi}_{ot}", tag="ps")
                for fb in range(FT):
                    nc.tensor.matmul(
                        ps,
                        lhsT=kb[fb][:, ot * P : (ot + 1) * P],
                        rhs=yt[:, fb, :],
                        start=(fb == 0),
                        stop=(fb == FT - 1),
                    )
                bias_ap = blob[:, ot : ot + 1]
                scale_ap = blob[:, OT + ot : OT + ot + 1]
                shift_ap = blob[:, 2 * OT + ot : 2 * OT + ot + 1]
                e = tmp.tile([P, panel], F32, name=f"r{rep}_e_{pi}_{ot}", tag="e")
                t0 = tmp.tile([P, panel], F32, name=f"r{rep}_t_{pi}_{ot}", tag="t")
                s = tmp.tile([P, panel], F32, name=f"r{rep}_s_{pi}_{ot}", tag="s")
                fin = tmp.tile([P, panel], BF16, name=f"r{rep}_f_{pi}_{ot}", tag="f")
                nc.scalar.activation(e, ps, ACTF.Exp, bias=bias_ap)
                nc.scalar.activation(t0, ps, ACTF.Relu, bias=bias_ap)
                # elu(zb) + 1 = min(exp(zb), relu(zb) + 1)   (exact identity)
                nc.vector.scalar_tensor_tensor(
                    s, in0=t0, scalar=1.0, in1=e, op0=ALU.add, op1=ALU.min
                )
                # fin = s*scale + (shift - scale) = elu*scale + shift
                nc.vector.tensor_scalar(
                    fin, s, scale_ap, shift_ap, op0=ALU.mult, op1=ALU.add
                )
                nc.sync.dma_start(outT_d[ot * P : (ot + 1) * P, r0 : r0 + panel], fin)
    nc.compile()
    return nc


def _host_prep(x, adjacency, kern, bias, gamma, beta, moving_mean, moving_var,
               n_cores=N_CORES):
    """Host-side prep: normalized adjacency mix, transpose/tile/cast to bf16.

    Returns (yt_per_core, kern_bf16, blob)."""
    B, C_, F = x.shape
    O = kern.shape[1]
    assert C_ == C
    bl = B // n_cores
    rows = bl * C
    FT, OT = F // P, O // P

    A = np.asarray(adjacency, np.float32)
    deg = np.maximum(np.abs(A).sum(axis=1, keepdims=True), 1e-8)
    dinv = deg ** -0.5
    na = A * dinv * dinv.T + np.eye(C, dtype=np.float32)  # [10,10]

    x_np = np.asarray(x, np.float32)
    # y[i, b, f] = sum_j na[i,j] x[b,j,f]  -- one sgemm [10,10]@[10,B*F]
    y_ibf = np.tensordot(na, x_np, axes=(1, 1)).astype(NP_BF16)  # [C, B, F]

    yt_per_core = []
    for c in range(n_cores):
        yc = y_ibf[:, c * bl : (c + 1) * bl, :]           # [C, bl, F]
        # yt[p, fb, b*C + i] = y[i, b, fb*128+p]
        yt = np.ascontiguousarray(
            yc.reshape(C, bl, FT, P).transpose(3, 2, 1, 0).reshape(P, FT, rows)
        )
        yt_per_core.append(yt)

    kern_bf16 = np.ascontiguousarray(np.asarray(kern, np.float32).astype(NP_BF16))

    scale = np.asarray(gamma, np.float32) / np.sqrt(
        np.asarray(moving_var, np.float32) + BN_EPS
    )
    shift2 = (
        np.asarray(beta, np.float32)
        - np.asarray(moving_mean, np.float32) * scale
        - scale
    )
    blob = np.zeros((P, 3 * OT), np.float32)
    blob[:, 0:OT] = np.asarray(bias, np.float32).reshape(OT, P).T
    blob[:, OT : 2 * OT] = scale.reshape(OT, P).T
    blob[:, 2 * OT : 3 * OT] = shift2.reshape(OT, P).T
    return yt_per_core, kern_bf16, blob


def kernel(x, adjacency, kernel, bias, gamma, beta, moving_mean, moving_var):
    B, C_, F = x.shape
    O = kernel.shape[1]
    assert C_ == C
    assert B % N_CORES == 0
    bl = B // N_CORES
    rows = bl * C

    yt_per_core, kern_bf16, blob = _host_prep(
        x, adjacency, kernel, bias, gamma, beta, moving_mean, moving_var
    )

    nc = build_nc(rows, F, O)

    in_maps = []
    for c in range(N_CORES):
        in_maps.append({
            "yt": yt_per_core[c],
            "kern": kern_bf16,
            "blob": blob,
        })

    res = run_bass_kernel_spmd(nc, in_maps, core_ids=list(range(N_CORES)), trace=False)

    out = np.empty((B, C, O), np.float32)
    for c in range(N_CORES):
        outT = np.asarray(res.results[c]["outT"]).astype(np.float32)  # [O, rows]
        out[c * bl : (c + 1) * bl] = outT.T.reshape(bl, C, O)
    return out
